# revision 2
# baseline (speedup 1.0000x reference)
"""MoE FFN (SwiGLU, E=8 experts, top-2) + shared expert, expert-parallel
across 8 NeuronCores.

Strategy: core e owns expert e (dense compute over all T=4096 tokens,
combine weight c_te zeroes non-selected tokens) + 1/8 of the shared FFN
hidden dim. Gate is computed on every core in fp32 with that core's
expert permuted to column 0 (top-2 selection is permutation-invariant),
so each core extracts its own combine column without needing a core id.
Host sums the 8 partial outputs.

FFN matmuls run in bf16 (fp32 PSUM accumulation); gate in fp32.
"""
import numpy as np
from contextlib import ExitStack

D, E, T = 1024, 8, 4096
H, HP, NHM = 2752, 2816, 22        # expert hidden, padded, 128-tiles
HS, HSP, NHS = 344, 384, 3         # shared-expert hidden shard per core
NK = 8                             # D / 128 contraction tiles
TN = 512                           # token tile (free axis)
NTN = T // TN                      # 8
NTM = TN // 128                    # 4 sub-tiles of 128 tokens

_CACHE = {}


def _build_nc():
    import concourse.bass as bass
    import concourse.tile as tile
    from concourse import bacc, mybir

    f32 = mybir.dt.float32
    bf16 = mybir.dt.bfloat16
    ALU = mybir.AluOpType
    ACT = mybir.ActivationFunctionType
    AX = mybir.AxisListType

    nc = bacc.Bacc("TRN2", target_bir_lowering=False, debug=False, num_devices=8)

    xr = nc.dram_tensor("xr", [128, NTN, NK, TN], f32, kind="ExternalInput").ap()
    xbr = nc.dram_tensor("xbr", [128, NTN, NK, TN], bf16, kind="ExternalInput").ap()
    gwr = nc.dram_tensor("gwr", [128, NK, E], f32, kind="ExternalInput").ap()
    w1r = nc.dram_tensor("w1r", [128, NHM, NK, 128], bf16, kind="ExternalInput").ap()
    w3r = nc.dram_tensor("w3r", [128, NHM, NK, 128], bf16, kind="ExternalInput").ap()
    w2r = nc.dram_tensor("w2r", [128, NHM, 1024], bf16, kind="ExternalInput").ap()
    s1r = nc.dram_tensor("s1r", [128, NHS, NK, 128], bf16, kind="ExternalInput").ap()
    s3r = nc.dram_tensor("s3r", [128, NHS, NK, 128], bf16, kind="ExternalInput").ap()
    s2r = nc.dram_tensor("s2r", [128, NHS, 1024], bf16, kind="ExternalInput").ap()
    out = nc.dram_tensor("out", [T, D], f32, kind="ExternalOutput").ap()

    with tile.TileContext(nc) as tc, ExitStack() as ctx:
        wpool = ctx.enter_context(tc.tile_pool(name="w", bufs=1))
        cpool = ctx.enter_context(tc.tile_pool(name="c", bufs=1))

        w1sb = wpool.tile([128, NHM, NK, 128], bf16, tag="w1")
        w3sb = wpool.tile([128, NHM, NK, 128], bf16, tag="w3")
        w2sb = wpool.tile([128, NHM, 1024], bf16, tag="w2")
        s1sb = wpool.tile([128, NHS, NK, 128], bf16, tag="s1")
        s3sb = wpool.tile([128, NHS, NK, 128], bf16, tag="s3")
        s2sb = wpool.tile([128, NHS, 1024], bf16, tag="s2")
        gwsb = wpool.tile([128, NK, E], f32, tag="gw")
        c_all = cpool.tile([128, NTN * NTM], f32, tag="call")

        nc.sync.dma_start(w1sb[:], w1r[:])
        nc.sync.dma_start(w3sb[:], w3r[:])
        nc.sync.dma_start(w2sb[:], w2r[:])
        nc.sync.dma_start(s1sb[:], s1r[:])
        nc.sync.dma_start(s3sb[:], s3r[:])
        nc.sync.dma_start(s2sb[:], s2r[:])
        nc.sync.dma_start(gwsb[:], gwr[:])

        # ---- gate prologue: fp32 logits -> top-2 combine weight for col 0 ----
        with tc.tile_pool(name="gx", bufs=2) as gxp, \
             tc.tile_pool(name="gp", bufs=2, space="PSUM") as gpp, \
             tc.tile_pool(name="gt", bufs=2) as gtp:
            for tn in range(NTN):
                xg = gxp.tile([128, NK, TN], f32, tag="xg")
                nc.sync.dma_start(xg[:], xr[:, tn])
                for tm in range(NTM):
                    pl = gpp.tile([128, E], f32, tag="pl")
                    for kk in range(NK):
                        nc.tensor.matmul(
                            pl[:], xg[:, kk, tm * 128:(tm + 1) * 128],
                            gwsb[:, kk, :], start=(kk == 0), stop=(kk == NK - 1))
                    col = tn * NTM + tm
                    m1n = gtp.tile([128, 1], f32, tag="m1n")
                    nc.vector.tensor_reduce(m1n[:], pl[:], axis=AX.X, op=ALU.max, negate=True)
                    s = gtp.tile([128, E], f32, tag="s")
                    nc.scalar.activation(s[:], pl[:], ACT.Exp, bias=m1n[:], scale=1.0)
                    eq = gtp.tile([128, E], f32, tag="eq")
                    nc.vector.tensor_scalar(eq[:], s[:], 1.0, None, op0=ALU.is_ge)
                    s2 = gtp.tile([128, E], f32, tag="s2")
                    nc.vector.scalar_tensor_tensor(s2[:], eq[:], -2.0, s[:], op0=ALU.mult, op1=ALU.add)
                    m2 = gtp.tile([128, 1], f32, tag="m2")
                    nc.vector.tensor_reduce(m2[:], s2[:], axis=AX.X, op=ALU.max)
                    msk = gtp.tile([128, 1], f32, tag="msk")
                    nc.vector.tensor_scalar(msk[:], s[:, 0:1], m2[:], None, op0=ALU.is_ge)
                    den = gtp.tile([128, 1], f32, tag="den")
                    nc.vector.tensor_scalar_add(den[:], m2[:], 1.0)
                    rec = gtp.tile([128, 1], f32, tag="rec")
                    nc.vector.reciprocal(rec[:], den[:])
                    nc.vector.scalar_tensor_tensor(
                        c_all[:, col:col + 1], s[:, 0:1], rec[:], msk[:],
                        op0=ALU.mult, op1=ALU.mult)

        # ---- main FFN loop ----
        with tc.tile_pool(name="xb", bufs=1) as xbp, \
             tc.tile_pool(name="h", bufs=1) as hp, \
             tc.tile_pool(name="sh", bufs=1) as shp, \
             tc.tile_pool(name="sil", bufs=2) as silp, \
             tc.tile_pool(name="o", bufs=2) as op_, \
             tc.tile_pool(name="pfw", bufs=2, space="PSUM") as pfw, \
             tc.tile_pool(name="pyo", bufs=2, space="PSUM") as pyo:
            for tn in range(NTN):
                xb = xbp.tile([128, NK, TN], bf16, tag="xb")
                nc.sync.dma_start(xb[:], xbr[:, tn])

                hT = hp.tile([128, NHM, TN], bf16, tag="hT")
                for hm in range(NHM):
                    p1 = pfw.tile([128, TN], f32, tag="p1")
                    p3 = pfw.tile([128, TN], f32, tag="p3")
                    for kk in range(NK):
                        nc.tensor.matmul(p1[:], w1sb[:, hm, kk, :], xb[:, kk, :],
                                         start=(kk == 0), stop=(kk == NK - 1))
                    for kk in range(NK):
                        nc.tensor.matmul(p3[:], w3sb[:, hm, kk, :], xb[:, kk, :],
                                         start=(kk == 0), stop=(kk == NK - 1))
                    sil = silp.tile([128, TN], f32, tag="sil")
                    nc.scalar.activation(sil[:], p1[:], ACT.Silu)
                    nc.vector.tensor_mul(hT[:, hm, :], sil[:], p3[:])

                shT = shp.tile([128, NHS, TN], bf16, tag="shT")
                for hs in range(NHS):
                    p1 = pfw.tile([128, TN], f32, tag="p1")
                    p3 = pfw.tile([128, TN], f32, tag="p3")
                    for kk in range(NK):
                        nc.tensor.matmul(p1[:], s1sb[:, hs, kk, :], xb[:, kk, :],
                                         start=(kk == 0), stop=(kk == NK - 1))
                    for kk in range(NK):
                        nc.tensor.matmul(p3[:], s3sb[:, hs, kk, :], xb[:, kk, :],
                                         start=(kk == 0), stop=(kk == NK - 1))
                    sil = silp.tile([128, TN], f32, tag="sil")
                    nc.scalar.activation(sil[:], p1[:], ACT.Silu)
                    nc.vector.tensor_mul(shT[:, hs, :], sil[:], p3[:])

                for tm in range(NTM):
                    osb = op_.tile([128, 1024], f32, tag="osb")
                    col = tn * NTM + tm
                    for dn in range(2):
                        yo = pyo.tile([128, 512], f32, tag="yo")
                        for hk in range(NHM):
                            nc.tensor.matmul(
                                yo[:], hT[:, hk, tm * 128:(tm + 1) * 128],
                                w2sb[:, hk, dn * 512:(dn + 1) * 512],
                                start=(hk == 0), stop=(hk == NHM - 1))
                        ys = pyo.tile([128, 512], f32, tag="ys")
                        for hs in range(NHS):
                            nc.tensor.matmul(
                                ys[:], shT[:, hs, tm * 128:(tm + 1) * 128],
                                s2sb[:, hs, dn * 512:(dn + 1) * 512],
                                start=(hs == 0), stop=(hs == NHS - 1))
                        osl = osb[:, dn * 512:(dn + 1) * 512]
                        nc.vector.tensor_scalar(osl, yo[:], c_all[:, col:col + 1],
                                                None, op0=ALU.mult)
                        nc.vector.tensor_add(osl, osl, ys[:])
                    t0 = tn * TN + tm * 128
                    nc.sync.dma_start(out[t0:t0 + 128, :], osb[:])

    nc.compile()
    return nc


def _prep_inputs(x, gate_w, w1, w3, w2, sw1, sw3, sw2):
    import ml_dtypes
    bf16 = ml_dtypes.bfloat16

    xf = np.ascontiguousarray(x.reshape(T, D).astype(np.float32))
    # [128p, NTN, NK, TN]: X[p,tn,kk,c] = xf[tn*TN+c, kk*128+p]
    xr = np.ascontiguousarray(xf.reshape(NTN, TN, NK, 128).transpose(3, 0, 2, 1))
    xbr = np.ascontiguousarray(xr.astype(bf16))

    def pad_rows(a, n):
        return np.concatenate([a, np.zeros((n - a.shape[0],) + a.shape[1:], a.dtype)], 0)

    in_maps = []
    for e in range(E):
        perm = [(e + j) % E for j in range(E)]
        gwr = np.ascontiguousarray(
            gate_w[perm].T.reshape(NK, 128, E).transpose(1, 0, 2).astype(np.float32))

        w1p = pad_rows(np.asarray(w1[e], np.float32), HP)   # [HP, D]
        w3p = pad_rows(np.asarray(w3[e], np.float32), HP)
        w2p = np.asarray(w2[e], np.float32)                 # [D, H]
        w2p = np.concatenate([w2p, np.zeros((D, HP - H), np.float32)], 1)

        w1r = np.ascontiguousarray(
            w1p.T.reshape(NK, 128, NHM, 128).transpose(1, 2, 0, 3).astype(bf16))
        w3r = np.ascontiguousarray(
            w3p.T.reshape(NK, 128, NHM, 128).transpose(1, 2, 0, 3).astype(bf16))
        w2r = np.ascontiguousarray(
            w2p.T.reshape(NHM, 128, D).transpose(1, 0, 2).astype(bf16))

        s1p = pad_rows(np.asarray(sw1[e * HS:(e + 1) * HS], np.float32), HSP)
        s3p = pad_rows(np.asarray(sw3[e * HS:(e + 1) * HS], np.float32), HSP)
        s2p = np.asarray(sw2[:, e * HS:(e + 1) * HS], np.float32)
        s2p = np.concatenate([s2p, np.zeros((D, HSP - HS), np.float32)], 1)

        s1rr = np.ascontiguousarray(
            s1p.T.reshape(NK, 128, NHS, 128).transpose(1, 2, 0, 3).astype(bf16))
        s3rr = np.ascontiguousarray(
            s3p.T.reshape(NK, 128, NHS, 128).transpose(1, 2, 0, 3).astype(bf16))
        s2rr = np.ascontiguousarray(
            s2p.T.reshape(NHS, 128, D).transpose(1, 0, 2).astype(bf16))

        in_maps.append({
            "xr": xr, "xbr": xbr, "gwr": gwr,
            "w1r": w1r, "w3r": w3r, "w2r": w2r,
            "s1r": s1rr, "s3r": s3rr, "s2r": s2rr,
        })
    return in_maps


def _run(in_maps, trace=False):
    from concourse.bass_utils import run_bass_kernel_spmd
    if "nc" not in _CACHE:
        _CACHE["nc"] = _build_nc()
    nc = _CACHE["nc"]
    res = run_bass_kernel_spmd(nc, in_maps, list(range(E)), trace=trace)
    return res


def kernel(x, gate_w, w1, w3, w2, sw1, sw3, sw2):
    in_maps = _prep_inputs(x, gate_w, w1, w3, w2, sw1, sw3, sw2)
    res = _run(in_maps)
    total = np.zeros((T, D), np.float32)
    for r in res.results:
        total += r["out"]
    return total.reshape(x.shape)
